# revision 1
# baseline (speedup 1.0000x reference)
"""StyleGAN2-style modulated 3x3 conv (B=16, C=128, H=W=128) on 8 TRN2 NeuronCores.

Sharding: data-parallel over batch. Each core gets 2 samples, computes its own
modulated weights, and runs the grouped conv locally as 9 accumulated
float32r matmuls per 4-row output tile (contraction over input channels = the
128 partition dim).
"""

import math
from itertools import product

import numpy as np

import concourse.bacc as bacc
import concourse.bass as bass
import concourse.mybir as mybir
import concourse.tile as tile
from concourse.bass_utils import run_bass_kernel_spmd
from concourse.masks import make_identity

B, C, H, W = 16, 128, 128, 128
KK = 3
EPS = 1e-8
N_CORES = 8
S = B // N_CORES          # samples per core
WP = W + 2                # zero-padded width
RPT = 4                   # output rows per PSUM tile
NFREE = RPT * W           # 512 = one PSUM bank of fp32
BH = 32                   # output rows per input band tile
NB = H // BH              # bands per sample
BROWS = BH + 2            # band buffer rows (1-row halo each side)
GT = 4                    # PSUM tiles per output store DMA (16 rows = 1 MB)

FP32 = mybir.dt.float32
FP32R = mybir.dt.float32r


def build_bass() -> bass.Bass:
    nc = bacc.Bacc(None)
    x_d = nc.dram_tensor("x", [S, C, H, W], FP32R, kind="ExternalInput")
    style_d = nc.dram_tensor("style", [S, C], FP32, kind="ExternalInput")
    w_d = nc.dram_tensor("weight", [C, C, KK, KK], FP32, kind="ExternalInput")
    out_d = nc.dram_tensor("out", [S, C, H, W], FP32, kind="ExternalOutput")

    with tile.TileContext(nc) as tc:
        with (
            tc.tile_pool(name="const", bufs=1) as const_pool,
            tc.tile_pool(name="wpool", bufs=1) as wpool,
            tc.tile_pool(name="xpool", bufs=4) as xpool,
            tc.tile_pool(name="opool", bufs=3) as opool,
            tc.tile_pool(name="dram", bufs=1, space="DRAM") as dram_pool,
            tc.tile_pool(name="psum_conv", bufs=4, space="PSUM") as psum_conv,
            tc.tile_pool(name="psum_misc", bufs=1, space="PSUM") as psum_misc,
        ):
            ident = const_pool.tile([128, 128], FP32)
            make_identity(nc, ident)
            zeros = const_pool.tile([128, WP], FP32)
            nc.vector.memset(zeros[:], 0.0)

            # ---- weight prep (shared by both samples) ----
            # Wt[o, i*9+k] : contiguous DMA of the raw weight
            Wt = wpool.tile([C, C * KK * KK], FP32)
            nc.sync.dma_start(Wt[:], w_d[:].rearrange("o i kh kw -> o (i kh kw)"))

            wmax = wpool.tile([C, 1], FP32)
            nc.vector.tensor_reduce(
                wmax[:], Wt[:], axis=mybir.AxisListType.X,
                op=mybir.AluOpType.max, apply_absolute_value=True,
            )
            winv = wpool.tile([C, 1], FP32)
            nc.vector.reciprocal(winv[:], wmax[:])
            nc.vector.tensor_scalar_mul(winv[:], winv[:], 1.0 / math.sqrt(C * KK * KK))
            # wn[o, i*9+k] = Wt * winv[o]
            wn = wpool.tile([C, C * KK * KK], FP32)
            nc.vector.tensor_scalar_mul(wn[:], Wt[:], winv[:])

            # transpose to wn_t[i, k*128+o] (9 PE transposes of 128x128)
            wn_t = wpool.tile([C, KK * KK * C], FP32)
            wn_koi = wn[:].rearrange("o (i k) -> o k i", k=KK * KK)
            for k in range(KK * KK):
                pt = psum_misc.tile([128, 128], FP32, name=f"pt{k}", tag="pt")
                nc.tensor.transpose(pt[:], wn_koi[:, k, :], ident[:])
                nc.vector.tensor_copy(wn_t[:, k * C:(k + 1) * C], pt[:])

            # qt[i, o] = sum_k wn_t[i, k, o]^2   (for the demod matvec)
            wsq = wpool.tile([C, KK * KK * C], FP32)
            nc.vector.tensor_mul(wsq[:], wn_t[:], wn_t[:])
            qt = wpool.tile([C, C], FP32)
            nc.vector.tensor_reduce(
                qt[:], wsq[:].rearrange("i (k o) -> i o k", k=KK * KK),
                axis=mybir.AxisListType.X, op=mybir.AluOpType.add,
            )

            # ---- style prep: cols 0..C-1 = s/||s||inf, cols C..2C-1 = square ----
            srow = wpool.tile([S, 2 * C], FP32)
            nc.sync.dma_start(srow[:, 0:C], style_d[:])
            smax = wpool.tile([S, 1], FP32)
            nc.vector.tensor_reduce(
                smax[:], srow[:, 0:C], axis=mybir.AxisListType.X,
                op=mybir.AluOpType.max, apply_absolute_value=True,
            )
            sinv = wpool.tile([S, 1], FP32)
            nc.vector.reciprocal(sinv[:], smax[:])
            nc.vector.tensor_scalar_mul(srow[:, 0:C], srow[:, 0:C], sinv[:])
            nc.vector.tensor_mul(srow[:, C:2 * C], srow[:, 0:C], srow[:, 0:C])

            # bounce through DRAM to get the per-partition (column) layout:
            # scol[c, h*S+b] = srow[b, h*C+c]  -> cols [s_b0, s_b1, s2_b0, s2_b1]
            s_dram = dram_pool.tile([S, 2 * C], FP32)
            nc.sync.dma_start(s_dram[:], srow[:])
            scol = wpool.tile([C, 2 * S], FP32)
            nc.sync.dma_start(scol[:, 0:S], s_dram[:, 0:C].rearrange("b c -> c b"))
            nc.sync.dma_start(scol[:, S:2 * S], s_dram[:, C:2 * C].rearrange("b c -> c b"))

            # coe[o, b] = 1/sqrt(sum_i qt[i,o]*s2[i,b] + eps)
            ps_coe = psum_misc.tile([C, S], FP32, tag="ps_coe")
            nc.tensor.matmul(ps_coe[:], qt[:], scol[:, S:2 * S], start=True, stop=True)
            eps_tile = wpool.tile([C, 1], FP32)
            nc.vector.memset(eps_tile[:], EPS)
            coe = wpool.tile([C, S], FP32)
            nc.scalar.activation(
                coe[:], ps_coe[:], mybir.ActivationFunctionType.Sqrt, bias=eps_tile[:],
            )
            nc.vector.reciprocal(coe[:], coe[:])

            # ---- per-sample modulated weights + conv ----
            for b in range(S):
                wmod = wpool.tile([C, KK * KK * C], FP32R, name=f"wmod{b}", tag=f"wmod{b}")
                nc.vector.tensor_scalar_mul(wmod[:], wn_t[:], scol[:, b:b + 1])

                for bi in range(NB):
                    r0 = bi * BH  # first output row of this band
                    # band buffer row j holds data row r0-1+j (rows -1/H are zero)
                    band = xpool.tile([C, BROWS, WP], FP32R, name="band", tag="band")
                    nc.vector.tensor_copy(band[:, :, 0], zeros[:, 0:BROWS])
                    nc.vector.tensor_copy(band[:, :, WP - 1], zeros[:, 0:BROWS])
                    lo = max(r0 - 1, 0)
                    hi = min(r0 + BH, H - 1)
                    j0 = lo - (r0 - 1)
                    if bi == 0:
                        nc.vector.tensor_copy(band[:, 0, :], zeros[:])
                    if bi == NB - 1:
                        nc.vector.tensor_copy(band[:, BROWS - 1, :], zeros[:])
                    nc.sync.dma_start(
                        band[:, j0:j0 + (hi - lo + 1), 1:W + 1],
                        x_d[b, :, lo:hi + 1, :],
                    )

                    for g in range(BH // (GT * RPT)):  # output groups in band
                        gy = g * GT * RPT  # local row offset of group
                        ot = opool.tile([C, GT * RPT, W], FP32, name="ot", tag="ot")
                        for u in range(GT):
                            yl = gy + u * RPT  # local output row offset
                            ps = psum_conv.tile([C, NFREE], FP32, name="ps", tag="ps")
                            for idx, (dy, dx) in enumerate(product(range(KK), range(KK))):
                                k = dy * KK + dx
                                nc.tensor.matmul(
                                    ps[:],
                                    wmod[:, k * C:(k + 1) * C],
                                    band[:, yl + dy:yl + dy + RPT, dx:dx + W],
                                    start=(idx == 0),
                                    stop=(idx == KK * KK - 1),
                                )
                            nc.vector.tensor_scalar_mul(
                                ot[:, u * RPT:(u + 1) * RPT, :],
                                ps[:].rearrange("c (r w) -> c r w", r=RPT),
                                coe[:, b:b + 1],
                            )
                        nc.sync.dma_start(
                            out_d[b, :, r0 + gy:r0 + gy + GT * RPT, :], ot[:],
                        )

    nc.compile()
    return nc


_CACHED = {}


def kernel(x: np.ndarray, style: np.ndarray, weight: np.ndarray, trace: bool = False):
    x = np.ascontiguousarray(x, dtype=np.float32)
    style = np.ascontiguousarray(style, dtype=np.float32)
    weight = np.ascontiguousarray(weight, dtype=np.float32)

    if "nc" not in _CACHED:
        _CACHED["nc"] = build_bass()
    nc = _CACHED["nc"]

    in_maps = [
        {
            "x": x[i * S:(i + 1) * S],
            "style": style[i * S:(i + 1) * S],
            "weight": weight,
        }
        for i in range(N_CORES)
    ]
    res = run_bass_kernel_spmd(
        nc, in_maps, core_ids=list(range(N_CORES)), trace=trace,
    )
    out = np.concatenate([r["out"] for r in res.results], axis=0)
    if trace:
        kernel.last_results = res
    return out

